# revision 15
# baseline (speedup 1.0000x reference)
"""Trainium2 Bass kernel for nn_MultiHeadAttention (decode-style, q_len=1).

Data-parallel over batch: 64 batches -> 8 cores x 8 batches.

Key algebraic restructuring (exact, exploits q_len == 1):
  scores[b,h,s] = (q Wq + bq)_h . (k Wk + bk)_h
                = k[b,s,:] . R_b[:,h] + const(b,h)        # const drops in softmax
     where R_b[d,h] = sum_{d'} Wk[d, h*64+d'] qh[b, h*64+d']
  out_concat[b,hd] = (sum_s p[b,h,s] v[b,s,:]) @ Wv[:,hd] + bv[hd]
so the big K/V projections (2 x 275 GFLOP) are never computed; instead
k and v are contracted directly (2 x 4.3 GFLOP) and the kernel becomes
HBM-bound on streaming k,v (128 MiB/core).

Perf structure (v2):
  - k and v stream via ONE 8 MiB SWDGE cast-DMA per batch each
    (f32 -> bf16), [128, 16, D] tiles, double buffered.  Few, huge
    descriptors keep the single SWDGE ring at HBM line rate.
  - Setup loads Wk/Wq in 2 MiB halves on the HWDGE ring (parallel with
    the k/v stream) and computes R with f32r matmuls; R is ready before
    batch 0's scores need it.
  - Tail computes OCT = Wv^T-blocks @ UT directly (64 N=128 matmuls),
    block-diagonal extract + bias via per-partition tensor_scalar adds,
    then the Wo projection.  Wv/Wo stream as bf16 cast-DMAs placed
    before the last v in the ring; the last v is split in 4 sub-DMAs so
    the final U starts as data lands.
"""

import numpy as np
from contextlib import ExitStack

import concourse.bass as bass
import concourse.tile as tile
from concourse import bacc, mybir
from concourse.bass_utils import run_bass_kernel_spmd

try:
    import axon_profile_shim
    axon_profile_shim.install()
except Exception:
    pass

N_CORES = 8
D = 1024
H = 16
DK = 64
F32 = mybir.dt.float32
F32R = mybir.dt.float32r
BF16 = mybir.dt.bfloat16
AX = mybir.AxisListType
ALU = mybir.AluOpType
ACTF = mybir.ActivationFunctionType


def _make_identity(nc, ap):
    nc.gpsimd.memset(ap, 0.0)
    nc.gpsimd.affine_select(
        out=ap, in_=ap, compare_op=ALU.not_equal, fill=1.0,
        base=0, pattern=[[-1, ap.shape[0]]], channel_multiplier=1,
    )


def build(BL=8, S=2048, n_cores=N_CORES):
    """Build + compile the per-core program. BL = local batches, S = seq len."""
    SC = S // 128          # 128-row s-subchunks (16)
    SG = S // 512          # 512-col score groups (4)
    J = S // 128           # rows per partition in a k/v tile (16)
    nc = bacc.Bacc("TRN2", target_bir_lowering=False, debug=False,
                   num_devices=n_cores)

    q_ext = nc.dram_tensor("q", [BL, D], F32, kind="ExternalInput").ap()
    k_ext = nc.dram_tensor("k", [BL * S, D], F32, kind="ExternalInput").ap()
    v_ext = nc.dram_tensor("v", [BL * S, D], F32, kind="ExternalInput").ap()
    Wq_ext = nc.dram_tensor("Wq", [D, D], F32, kind="ExternalInput").ap()
    Wk_ext = nc.dram_tensor("Wk", [D, D], F32, kind="ExternalInput").ap()
    Wv_ext = nc.dram_tensor("Wv", [D, D], F32, kind="ExternalInput").ap()
    Wo_ext = nc.dram_tensor("Wo", [D, D], F32, kind="ExternalInput").ap()
    bq_ext = nc.dram_tensor("bq", [D], F32, kind="ExternalInput").ap()
    bv_ext = nc.dram_tensor("bv", [D], F32, kind="ExternalInput").ap()
    bo_ext = nc.dram_tensor("bo", [D], F32, kind="ExternalInput").ap()
    y_ext = nc.dram_tensor("y", [BL, D], F32, kind="ExternalOutput").ap()

    with tile.TileContext(nc) as tc, ExitStack() as ctx:
        cpool = ctx.enter_context(tc.tile_pool(name="const", bufs=1))
        ident = cpool.tile([128, 128], F32)
        ident_bf = cpool.tile([128, 128], BF16)
        bo8 = cpool.tile([BL, D], F32)

        # persistent across whole kernel
        R_all = cpool.tile([128, 8, H * BL], BF16)
        UT_all = cpool.tile([128, 8, H, BL], BF16)
        OCT_sb = cpool.tile([128, 8, BL], BF16)
        bvT = cpool.tile([128, 8], F32)

        # ---------------- stream pools + k(0)/v(0) prefetch ----------------
        kpool = ctx.enter_context(tc.tile_pool(name="kpool", bufs=2))
        vrpool = ctx.enter_context(tc.tile_pool(name="vrpool", bufs=2))

        def load_k(b):
            # One 8 MiB cast-DMA: partition p holds rows J*p .. J*p+J-1
            # (64 KB contiguous f32 per partition -> bf16).  The s-
            # permutation s = J*p + j is applied identically to k and v;
            # softmax/U are order-invariant in s.
            kbf = kpool.tile([128, J, D], BF16, tag="kbf", name="kbf")
            r0 = b * S
            nc.gpsimd.dma_start(
                kbf[:], k_ext[r0:r0 + S, :].rearrange("(p j) d -> p j d", p=128))
            return kbf

        def load_v(b, nsplit=1):
            vr = vrpool.tile([128, J, D], BF16, tag="vr", name="vr")
            r0 = b * S
            src = v_ext[r0:r0 + S, :].rearrange("(p j) d -> p j d", p=128)
            step = J // nsplit
            for i in range(nsplit):
                nc.gpsimd.dma_start(
                    vr[:, i * step:(i + 1) * step, :],
                    src[:, i * step:(i + 1) * step, :])
            return vr

        k_tiles = load_k(0)
        _make_identity(nc, ident[:])
        nc.vector.tensor_copy(ident_bf[:], ident[:])
        nc.sync.dma_start(bo8[:], bo_ext.unsqueeze(0).broadcast_to([BL, D]))

        # ---------------- setup: qh^T, Wk^T, R, bv^T ----------------
        # Weight loads ride the SWDGE (gpsimd) ring, explicitly ordered
        # k0 -> Wq -> Wk -> v0 -> k1: the HWDGE ring is starved to ~1%
        # while 8 MiB SWDGE stream DMAs are in flight, so putting the
        # weights on the same FIFO ring is the only way to bound their
        # arrival.  512 KiB stages interleave with their consumers so
        # slot-reuse WAR deps clear before the ring reaches them.
        with tc.tile_pool(name="wsetup", bufs=1) as wpool, \
             tc.tile_pool(name="wstage", bufs=4) as wstage, \
             tc.tile_pool(name="spsum", bufs=1, space="PSUM") as spsum:
            Q = wpool.tile([BL, D], F32)
            nc.sync.dma_start(Q[:], q_ext[:])
            bq8 = wpool.tile([BL, D], F32)
            nc.sync.dma_start(bq8[:], bq_ext.unsqueeze(0).broadcast_to([BL, D]))

            # Q^T then qh = Q @ Wq + bq
            qtp = spsum.tile([128, 128], F32, tag="sp128")
            for i in range(8):
                nc.tensor.transpose(qtp[:, i * BL:(i + 1) * BL],
                                    Q[:, i * 128:(i + 1) * 128], ident[:BL, :BL])
            QT_sb = wpool.tile([128, 8 * BL], F32)
            nc.vector.tensor_copy(QT_sb[:], qtp[:, :8 * BL])

            qhp = spsum.tile([BL, D], F32, tag="qhp")
            for a in range(8):
                wq_st = wstage.tile([128, 1, D], F32, tag="w_st", name="wq_st")
                nc.gpsimd.dma_start(
                    wq_st[:],
                    Wq_ext[a * 128:(a + 1) * 128, :].unsqueeze(1)
                    .rearrange("p a d -> p a d"))
                for n in range(2):
                    nc.tensor.matmul(qhp[:, n * 512:(n + 1) * 512],
                                     QT_sb[:, a * BL:(a + 1) * BL],
                                     wq_st[:, 0, n * 512:(n + 1) * 512],
                                     start=(a == 0), stop=(a == 7))
            qh_sb = wpool.tile([BL, D], F32)
            nc.vector.tensor_add(qh_sb[:], qhp[:], bq8[:])
            qtp2 = spsum.tile([128, 128], F32, tag="sp128")
            for m in range(8):
                nc.tensor.transpose(qtp2[:, m * BL:(m + 1) * BL],
                                    qh_sb[:, m * 128:(m + 1) * 128],
                                    ident[:BL, :BL])
            qhT_sb = wpool.tile([128, 8 * BL], F32)  # [p, m*BL + b]
            nc.vector.tensor_copy(qhT_sb[:], qtp2[:, :8 * BL])

            # Block-diagonal qh for ALL batches:
            # qblk_c[p, b*16+h] = qh_b[c*128+p] if h == head(c*128+p) else 0
            zeros32 = wpool.tile([128, H * BL], F32)
            nc.vector.memset(zeros32[:], 0.0)
            qblk = [wpool.tile([128, H * BL], F32R, tag=f"qblk{c}", name=f"qblk{c}")
                    for c in range(8)]
            for c in range(8):
                nc.vector.tensor_copy(qblk[c][:], zeros32[:])
                lo = qblk[c][0:64, :].rearrange("p (b h) -> p b h", h=H)
                hi = qblk[c][64:128, :].rearrange("p (b h) -> p b h", h=H)
                nc.vector.tensor_copy(
                    lo[:, :, 2 * c:2 * c + 1],
                    qhT_sb[0:64, c * BL:(c + 1) * BL].unsqueeze(2))
                nc.vector.tensor_copy(
                    hi[:, :, 2 * c + 1:2 * c + 2],
                    qhT_sb[64:128, c * BL:(c + 1) * BL].unsqueeze(2))
            # Wk in 512 KiB eighth-loads -> WkT via staged PE transposes
            WkT = [wpool.tile([128, D], F32R, tag=f"wkt{c}", name=f"wkt{c}")
                   for c in range(8)]
            for a in range(8):
                wk_st = wstage.tile([128, 1, D], F32, tag="w_st", name="wk_st")
                nc.gpsimd.dma_start(
                    wk_st[:],
                    Wk_ext[a * 128:(a + 1) * 128, :].unsqueeze(1)
                    .rearrange("p a d -> p a d"))
                wp = spsum.tile([128, D], F32, tag="wtp", name="wp")
                for c in range(8):
                    nc.tensor.transpose(wp[:, c * 128:(c + 1) * 128],
                                        wk_st[:, 0, c * 128:(c + 1) * 128],
                                        ident[:])
                for c in range(8):
                    nc.vector.tensor_copy(WkT[c][:, a * 128:(a + 1) * 128],
                                          wp[:, c * 128:(c + 1) * 128])

            # v0 and k1 enter the SWDGE ring right after the weights
            v_tiles0 = load_v(0)
            k1_pre = load_k(1) if BL > 1 else None

            RT_sb = wpool.tile([H * BL, D], F32)  # [b*16+h, d]
            for n in range(2):
                rtp = spsum.tile([H * BL, 512], F32, tag="rtp", name="rtp")
                for c in range(8):
                    nc.tensor.matmul(rtp[:], qblk[c][:],
                                     WkT[c][:, n * 512:(n + 1) * 512],
                                     start=(c == 0), stop=(c == 7))
                nc.vector.tensor_copy(RT_sb[:, n * 512:(n + 1) * 512], rtp[:])
            for j in range(8):
                rp = spsum.tile([128, 128], F32, tag="sp128", name="rp")
                nc.tensor.transpose(rp[:, :H * BL], RT_sb[:, j * 128:(j + 1) * 128],
                                    ident[:H * BL, :H * BL])
                nc.vector.tensor_copy(R_all[:, j, :], rp[:, :H * BL])

            # bv^T: bvT[p, a] = bv[a*128 + p]
            bv8r = wpool.tile([8, 128], F32)
            nc.sync.dma_start(bv8r[:], bv_ext.rearrange("(a p) -> a p", a=8))
            bvp = spsum.tile([128, 128], F32, tag="sp128", name="bvp")
            nc.tensor.transpose(bvp[:, :8], bv8r[:], ident[:8, :8])
            nc.vector.tensor_copy(bvT[:], bvp[:, :8])

        # stream-only pools, created after setup frees its SBUF
        ktpool = ctx.enter_context(tc.tile_pool(name="ktpool", bufs=3))
        epool = ctx.enter_context(tc.tile_pool(name="epool", bufs=2))
        e1pool = ctx.enter_context(tc.tile_pool(name="e1pool", bufs=1))
        etpool = ctx.enter_context(tc.tile_pool(name="etpool", bufs=2))
        upool = ctx.enter_context(tc.tile_pool(name="upool", bufs=1))
        tailw = ctx.enter_context(tc.tile_pool(name="tailw", bufs=1))
        Wv_sb = tailw.tile([128, 8, D], BF16, tag="wv", name="wv")
        Wo_r = tailw.tile([128, 8, D], BF16, tag="wor", name="wor")

        # ---------------- stream phase ----------------
        stream_psum = ExitStack()
        tpp = stream_psum.enter_context(tc.tile_pool(name="tpp", bufs=1, space="PSUM"))
        ktp = stream_psum.enter_context(tc.tile_pool(name="ktp", bufs=3, space="PSUM"))
        scp = stream_psum.enter_context(tc.tile_pool(name="scp", bufs=2, space="PSUM"))
        upp = stream_psum.enter_context(tc.tile_pool(name="upp", bufs=1, space="PSUM"))

        for b in range(BL):
            E_b = e1pool.tile([H, S], BF16, tag="E")
            den4 = epool.tile([H, SG], F32, tag="den4")
            if b == BL - 2 and BL > 2:
                # last k one ring-slot early: batch BL-1's scores can then
                # finish before its v lands, shrinking the trailing tail
                k_next = load_k(b + 1)
            v_tiles = v_tiles0 if b == 0 else load_v(b, nsplit=4 if b == BL - 1 else 1)
            if b + 1 < BL and not (b == BL - 2 and BL > 2):
                k_next = k1_pre if b == 0 else load_k(b + 1)
            if b == max(0, BL - 3):
                # bf16 cast-loads of the tail weights ride the SWDGE ring
                # ahead of the last v
                nc.gpsimd.dma_start(
                    Wv_sb[:], Wv_ext.rearrange("(a p) d -> p a d", p=128))
            if b == max(0, BL - 2):
                nc.gpsimd.dma_start(
                    Wo_r[:], Wo_ext.rearrange("(a p) d -> p a d", p=128))

            for g in range(SG):
                kt4 = ktpool.tile([128, 8, 512], BF16, tag="kt4")
                for j2 in range(4):
                    j = g * 4 + j2
                    for half in range(2):
                        tp = ktp.tile([128, 512], BF16, tag="ktp", name="tp")
                        for d4 in range(4):
                            dj = half * 4 + d4
                            nc.tensor.transpose(tp[:, d4 * 128:(d4 + 1) * 128],
                                                k_tiles[:, j, dj * 128:(dj + 1) * 128],
                                                ident_bf[:])
                        nc.vector.tensor_copy(
                            kt4[:, half * 4:(half + 1) * 4, j2 * 128:(j2 + 1) * 128],
                            tp[:].rearrange("p (a b) -> p a b", a=4))
                sc = scp.tile([H, 512], F32, tag="sc")
                for dj in range(8):
                    nc.tensor.matmul(sc[:], R_all[:, dj, b * H:(b + 1) * H],
                                     kt4[:, dj, :],
                                     start=(dj == 0), stop=(dj == 7))
                nc.scalar.activation(E_b[:, g * 512:(g + 1) * 512], sc[:],
                                     ACTF.Exp, scale=0.125,
                                     accum_out=den4[:, g:g + 1])

            den = epool.tile([H, 1], F32, tag="den")
            nc.vector.tensor_reduce(den[:], den4[:], axis=AX.X, op=ALU.add)
            rden = epool.tile([H, 1], F32, tag="rden")
            nc.vector.reciprocal(rden[:], den[:])

            ET_b = etpool.tile([128, SC, H], BF16, tag="ET")
            gsz = min(8, SC)
            for tg in range(SC // gsz):
                sp = tpp.tile([128, 128], BF16, tag="tpE", name="spE")
                for i in range(gsz):
                    t = tg * gsz + i
                    nc.tensor.transpose(sp[:, i * H:(i + 1) * H],
                                        E_b[:, t * 128:(t + 1) * 128],
                                        ident_bf[:H, :H])
                nc.vector.tensor_copy(
                    ET_b[:, tg * gsz:(tg + 1) * gsz, :],
                    sp[:, :gsz * H].rearrange("p (t h) -> p t h", t=gsz))

            up = upp.tile([H, D], F32, tag="up")
            for t in range(SC):
                for n in range(2):
                    nc.tensor.matmul(up[:, n * 512:(n + 1) * 512],
                                     ET_b[:, t, :],
                                     v_tiles[:, t, n * 512:(n + 1) * 512],
                                     start=(t == 0), stop=(t == SC - 1))
            U_sb = upool.tile([H, D], BF16, tag="U")
            nc.vector.tensor_scalar_mul(U_sb[:], up[:], rden[:])

            sp = tpp.tile([128, 128], BF16, tag="tpE")
            for jc in range(8):
                nc.tensor.transpose(sp[:, jc * H:(jc + 1) * H],
                                    U_sb[:, jc * 128:(jc + 1) * 128],
                                    ident_bf[:H, :H])
            nc.vector.tensor_copy(
                UT_all[:, :, :, b],
                sp[:, :8 * H].rearrange("p (j h) -> p j h", j=8))
            if b + 1 < BL:
                k_tiles = k_next

        # ---------------- tail: out-projection ----------------
        stream_psum.close()
        with tc.tile_pool(name="fin", bufs=1) as fpool, \
             tc.tile_pool(name="fpsum", bufs=1, space="PSUM") as fpsum:
            # OCT_full[e, (h,b)] = sum_d Wv[d, e] * UT[d, (h,b)], per e-block
            octp = fpsum.tile([128, 8, H * BL], F32, tag="octp")
            for ec in range(8):
                for jc in range(8):
                    nc.tensor.matmul(
                        octp[:, ec, :],
                        Wv_sb[:, jc, ec * 128:(ec + 1) * 128],
                        UT_all[:, jc, :, :].rearrange("p h b -> p (h b)"),
                        start=(jc == 0), stop=(jc == 7))
            # extract block-diagonal (head-of-e match) + bias, cast bf16
            for ec in range(8):
                h_lo, h_hi = 2 * ec, 2 * ec + 1
                nc.vector.tensor_scalar_add(
                    OCT_sb[0:64, ec, :],
                    octp[0:64, ec, h_lo * BL:(h_lo + 1) * BL],
                    bvT[0:64, ec:ec + 1])
                nc.vector.tensor_scalar_add(
                    OCT_sb[64:128, ec, :],
                    octp[64:128, ec, h_hi * BL:(h_hi + 1) * BL],
                    bvT[64:128, ec:ec + 1])

            yp = fpsum.tile([BL, D], F32, tag="yp")
            for n in range(2):
                for ec in range(8):
                    nc.tensor.matmul(yp[:, n * 512:(n + 1) * 512],
                                     OCT_sb[:, ec, :],
                                     Wo_r[:, ec, n * 512:(n + 1) * 512],
                                     start=(ec == 0), stop=(ec == 7))
            y_sb = fpool.tile([BL, D], F32)
            nc.vector.tensor_add(y_sb[:], yp[:], bo8[:])
            nc.vector.tensor_scalar_max(y_sb[:], y_sb[:], 0.0)
            nc.sync.dma_start(y_ext[:], y_sb[:])

    nc.compile()
    return nc


_built = {}


def _get_nc(BL, S):
    key = (BL, S)
    if key not in _built:
        _built[key] = build(BL, S)
    return _built[key]


def kernel(q, k, v, Wq, bq, Wk, bk, Wv, bv, Wo, bo, _trace=False):
    q = np.asarray(q, dtype=np.float32)
    k = np.asarray(k, dtype=np.float32)
    v = np.asarray(v, dtype=np.float32)
    B, S = k.shape[0], k.shape[1]
    BL = B // N_CORES
    nc = _get_nc(BL, S)

    shared = {
        "Wq": np.ascontiguousarray(Wq, dtype=np.float32),
        "Wk": np.ascontiguousarray(Wk, dtype=np.float32),
        "Wv": np.ascontiguousarray(Wv, dtype=np.float32),
        "Wo": np.ascontiguousarray(Wo, dtype=np.float32),
        "bq": np.ascontiguousarray(bq, dtype=np.float32),
        "bv": np.ascontiguousarray(bv, dtype=np.float32),
        "bo": np.ascontiguousarray(bo, dtype=np.float32),
    }
    in_maps = []
    for c in range(N_CORES):
        sl = slice(c * BL, (c + 1) * BL)
        in_maps.append({
            "q": np.ascontiguousarray(q[sl].reshape(BL, D)),
            "k": np.ascontiguousarray(k[sl].reshape(BL * S, D)),
            "v": np.ascontiguousarray(v[sl].reshape(BL * S, D)),
            **shared,
        })
    res = run_bass_kernel_spmd(nc, in_maps, list(range(N_CORES)), trace=_trace)
    out = np.concatenate([res.results[c]["y"] for c in range(N_CORES)], axis=0)
    if _trace:
        kernel._last_exec_time_ns = res.exec_time_ns
        kernel._last_profile = res.profile_json
    return out
